# revision 27
# baseline (speedup 1.0000x reference)
"""ConceptEmbedding kernel for 8 Trainium2 NeuronCores.

Data-parallel over batch (B=8 -> 1 batch per core). Per core:
  m[s,d]   = sum_c seq[s,c] * emb[c,d]      (PE, bf16 inputs, fp32 psum)
  cnt[s]   = sum_c seq[s,c]                 (fused: ones column appended to emb rhs)
  f        = m / max(cnt,1)                 (e_free; u8 seq scale cancels in the ratio)
  idx[s]   = argmax_k (f . cent_k - 0.5*|cent_k|^2)   == argmin_k ||f - cent_k||^2
  out      = 0.1*f + 0.9*cent[idx]

The session bottleneck is the ~50-70 MB/s axon tunnel, not the device, so the
wire format is minimized: seq ships 4-bit quantized, two values per byte
(round(15*x); the scale cancels in m/cnt; column c' of the packed byte holds
column c' in the low nibble and column c'+C/2 in the high nibble, so the
device unpack keeps the original column order), the embedding table ships u8
(round(255*e), the 1/255 folded into the reciprocal stage) and sharded
1/8-per-core (AllGather on device rebuilds the full table), centroids ship
fp32 sharded 1/8-per-core, the K-k tie-break ramp is generated on device with
iota, and the output returns as u8 (x255). Host-side casts run through
jax-cpu jits - plain numpy elementwise on this host is ~100x slower.
"""

import os as _os
import sys
from contextlib import ExitStack

sys.path.insert(0, "/opt/trn_rl_repo")

# The per-call jax.jit inside run_bass_via_pjrt is a fresh closure, so every
# call misses jax's in-memory executable cache and re-runs the neuronx hook
# (~0.5s). The persistent compilation cache turns that into a disk hit.
_os.environ.setdefault("JAX_COMPILATION_CACHE_DIR", "/tmp/jaxcache")
_os.environ.setdefault("JAX_PERSISTENT_CACHE_MIN_COMPILE_TIME_SECS", "0")
_os.environ.setdefault("JAX_PERSISTENT_CACHE_MIN_ENTRY_SIZE_BYTES", "0")

import numpy as np

import concourse.bass as bass
import concourse.mybir as mybir
import concourse.tile as tile
from concourse import bacc
from concourse.bass_utils import run_bass_kernel_spmd
from concourse.masks import make_identity

B, S, C, D, K = 8, 1024, 8192, 256, 512
FREEDOM = 0.1
P = 128
CT = C // P  # 64 c-tiles
ST = S // P  # 8 s-tiles
KT = K // P  # 4 k-tiles
DH = D // P  # 2 d-halves
NCORE = 8
CSH = C // NCORE  # 1024 emb rows per core on the wire
KSH = K // NCORE  # 64 centroid rows per core on the wire

fp32 = mybir.dt.float32
bf16 = mybir.dt.bfloat16
i32 = mybir.dt.int32
u8 = mybir.dt.uint8

_nc_cache = {}


def _ensure_jax_cache():
    """config.update is authoritative even if jax was imported before us."""
    if "jaxcache" in _nc_cache:
        return
    import jax

    try:
        jax.config.update("jax_compilation_cache_dir", "/tmp/jaxcache")
        jax.config.update("jax_persistent_cache_min_compile_time_secs", 0.0)
        jax.config.update("jax_persistent_cache_min_entry_size_bytes", 0)
    except Exception:
        pass
    _nc_cache["jaxcache"] = True


def _body(ctx, tc, nc, blob, centp, out):
    # blob[s, 0:C/2] = packed 4-bit seq row s; blob[s, C/2:C/2+D] = u8 emb
    # shard row s (seq and the per-core emb shard share S=CSH=1024 rows, so
    # they merge into one wire tensor - each extra input array costs ~80ms
    # of per-transfer overhead on the axon tunnel)
    mult = mybir.AluOpType.mult
    add = mybir.AluOpType.add
    is_ge = mybir.AluOpType.is_ge
    is_equal = mybir.AluOpType.is_equal
    AX = mybir.AxisListType.X

    const = ctx.enter_context(tc.tile_pool(name="const", bufs=1))
    dram = ctx.enter_context(tc.tile_pool(name="dram", bufs=1, space="DRAM"))
    natq_pool = ctx.enter_context(tc.tile_pool(name="natq", bufs=2))
    nat_pool = ctx.enter_context(tc.tile_pool(name="nat", bufs=2))
    seqT_pool = ctx.enter_context(tc.tile_pool(name="seqT", bufs=2))
    work = ctx.enter_context(tc.tile_pool(name="work", bufs=3))
    outp = ctx.enter_context(tc.tile_pool(name="outp", bufs=3))
    ps_t = ctx.enter_context(tc.tile_pool(name="ps_t", bufs=2, space="PSUM"))
    ps_m = ctx.enter_context(tc.tile_pool(name="ps_m", bufs=2, space="PSUM"))
    ps_g = ctx.enter_context(tc.tile_pool(name="ps_g", bufs=2, space="PSUM"))
    ps_f = ctx.enter_context(tc.tile_pool(name="ps_f", bufs=2, space="PSUM"))

    # ---------------- gather the sharded tables on device ----------------
    groups = [list(range(NCORE))]
    emb_in = dram.tile([CSH, D], u8)
    emb_full = dram.tile([C, D], u8)
    nc.gpsimd.dma_start(emb_in[:], blob[:, C // 2 : C // 2 + D])
    nc.gpsimd.collective_compute(
        "AllGather",
        mybir.AluOpType.bypass,
        replica_groups=groups,
        ins=[emb_in.opt()],
        outs=[emb_full.opt()],
    )
    cent_in = dram.tile([KSH, D], fp32)
    cent_full = dram.tile([K, D], fp32)
    nc.gpsimd.dma_start(cent_in[:], centp[:])
    nc.gpsimd.collective_compute(
        "AllGather",
        mybir.AluOpType.bypass,
        replica_groups=groups,
        ins=[cent_in.opt()],
        outs=[cent_full.opt()],
    )

    # ---------------- constants ----------------
    ident = const.tile([P, P], bf16)
    make_identity(nc, ident[:])
    ident_f = const.tile([P, P], fp32)
    make_identity(nc, ident_f[:])

    # emb_aug[p, t, 0:256] = round(255*emb[t*128+p, :]) as bf16 (u8 values are
    # exact in bf16; the 1/255 is folded into the reciprocal stage);
    # col 256 = 1.0 (row count)
    emb_u8_sb = const.tile([P, CT, D], u8)
    nc.gpsimd.dma_start(
        out=emb_u8_sb[:],
        in_=emb_full[:].rearrange("(t p) d -> p t d", p=P),
    )
    emb_aug = const.tile([P, CT, D + 1], bf16)
    nc.vector.tensor_copy(emb_aug[:, :, 0:D], emb_u8_sb[:])
    nc.vector.memset(emb_aug[:, :, D : D + 1], 1.0)

    # centroids natural fp32; transposed fp32 centT[d, k] (the scoring path
    # must be fp32: bf16 jitter exceeds the argmin margins)
    cent_nat = const.tile([P, KT, D], fp32)
    nc.sync.dma_start(cent_nat[:], cent_full[:].rearrange("(t p) d -> p t d", p=P))

    centT = const.tile([P, DH, K], fp32)
    for t in range(KT):
        pst = ps_t.tile([P, 4, P], fp32, tag="tp")
        for dh in range(DH):
            nc.tensor.matmul(
                pst[:, dh, :],
                lhsT=cent_nat[:, t, dh * P : (dh + 1) * P],
                rhs=ident_f[:],
                start=True,
                stop=True,
            )
        nc.any.tensor_copy(centT[:, :, t * P : (t + 1) * P], pst[:, 0:DH, :])

    # negh_mat[s, k] = -0.5 * |cent_k|^2 (same row in every partition, fp32)
    sq = const.tile([P, KT, D], fp32)
    nc.vector.tensor_tensor(sq[:], cent_nat[:], cent_nat[:], op=mult)
    negh_col = const.tile([P, KT], fp32)
    nc.vector.tensor_reduce(negh_col[:], sq[:], axis=AX, op=add)
    negh_cols = const.tile([P, KT], fp32)
    nc.vector.tensor_scalar_mul(negh_cols[:], negh_col[:], -0.5)
    psh = ps_g.tile([P, K], fp32, tag="g")
    for t in range(KT):
        nc.tensor.matmul(
            psh[:, t * P : (t + 1) * P],
            lhsT=negh_cols[:, t : t + 1].to_broadcast([P, P]),
            rhs=ident_f[:],
            start=True,
            stop=True,
        )
    negh_mat = const.tile([P, K], fp32)
    nc.vector.tensor_copy(negh_mat[:], psh[:])

    # rev[p, k] = K - k (same in every partition), via on-device iota
    rev_i = const.tile([P, K], i32)
    nc.gpsimd.iota(rev_i[:], pattern=[[-1, K]], base=K, channel_multiplier=0)
    rev_f = const.tile([P, K], fp32)
    nc.vector.tensor_copy(rev_f[:], rev_i[:])

    # ---------------- main loop over s-tiles ----------------
    band = mybir.AluOpType.bitwise_and
    shr = mybir.AluOpType.logical_shift_right
    for i in range(ST):
        natq = natq_pool.tile([P, C // 2], u8)  # packed 4-bit seq rows
        nc.gpsimd.dma_start(natq[:], blob[i * P : (i + 1) * P, 0 : C // 2])
        # unpack nibbles -> bf16 (exact for 0..15); low nibble = cols [0, C/2),
        # high nibble = cols [C/2, C)
        lo_u8 = nat_pool.tile([P, C // 2], u8, tag="lo8")
        hi_u8 = nat_pool.tile([P, C // 2], u8, tag="hi8")
        nc.vector.tensor_scalar(lo_u8[:], natq[:], 15, None, op0=band)
        nc.vector.tensor_scalar(hi_u8[:], natq[:], 4, None, op0=shr)
        nat = nat_pool.tile([P, C], bf16, tag="nat")
        nc.any.tensor_copy(nat[:, 0 : C // 2], lo_u8[:])
        nc.any.tensor_copy(nat[:, C // 2 : C], hi_u8[:])

        # transpose 64 c-tiles on PE: seqT[c_local, ct, s_local]
        seqT = seqT_pool.tile([P, CT, P], bf16)
        for g in range(CT // 4):
            pst = ps_t.tile([P, 4, P], fp32, tag="tp")
            for j in range(4):
                c = g * 4 + j
                nc.tensor.matmul(
                    pst[:, j, :],
                    lhsT=nat[:, c * P : (c + 1) * P],
                    rhs=ident[:],
                    start=True,
                    stop=True,
                )
            nc.any.tensor_copy(seqT[:, g * 4 : (g + 1) * 4, :], pst[:])

        # main accumulation: psm[s, 0:256] = (15*255)*m-ish, psm[s, 256] = 15*cnt;
        # the common 15 cancels in m/cnt, the 255 is divided out via recip255
        psm = ps_m.tile([P, D + 1], fp32)
        for c in range(CT):
            nc.tensor.matmul(
                psm[:],
                lhsT=seqT[:, c, :],
                rhs=emb_aug[:, c, :],
                start=(c == 0),
                stop=(c == CT - 1),
            )

        # cnt guard + reciprocal
        iszero = work.tile([P, 1], fp32)
        nc.vector.tensor_scalar(iszero[:], psm[:, D : D + 1], 0.0, None, op0=is_equal)
        cnt_adj = work.tile([P, 1], fp32)
        nc.vector.tensor_tensor(cnt_adj[:], psm[:, D : D + 1], iszero[:], op=add)
        recip = work.tile([P, 1], fp32)
        nc.vector.reciprocal(recip[:], cnt_adj[:])
        recip255 = work.tile([P, 1], fp32)
        nc.vector.tensor_scalar(recip255[:], recip[:], 1.0 / 255.0, None, op0=mult)

        # f = m / cnt / 255 (fp32 for the scoring path; /255 undoes the u8 emb)
        f_sb = work.tile([P, D], fp32)
        nc.vector.tensor_scalar(f_sb[:], psm[:, 0:D], recip255[:], None, op0=mult)

        # fT via PE transpose
        psf = ps_f.tile([P, DH, P], fp32)
        for dh in range(DH):
            nc.tensor.matmul(
                psf[:, dh, :],
                lhsT=f_sb[:, dh * P : (dh + 1) * P],
                rhs=ident_f[:],
                start=True,
                stop=True,
            )
        fT = work.tile([P, DH, P], fp32)
        nc.any.tensor_copy(fT[:], psf[:])

        # G[s, k] = f . cent_k, then add -0.5|cent_k|^2 on DVE (fp32 path)
        psg = ps_g.tile([P, K], fp32, tag="g")
        nc.tensor.matmul(psg[:], lhsT=fT[:, 0, :], rhs=centT[:, 0, :], start=True, stop=False)
        nc.tensor.matmul(psg[:], lhsT=fT[:, 1, :], rhs=centT[:, 1, :], start=False, stop=True)
        gsc = work.tile([P, K], fp32)
        nc.vector.tensor_tensor(gsc[:], psg[:], negh_mat[:], op=add)

        # argmax over k (first max index, matching jnp.argmin tie-break)
        mx = work.tile([P, 1], fp32)
        nc.vector.reduce_max(mx[:], gsc[:], axis=AX)
        eq = work.tile([P, K], bf16)
        nc.vector.tensor_scalar(eq[:], gsc[:], mx[:], None, op0=is_ge)
        val = work.tile([P, K], fp32)
        nc.vector.tensor_tensor(val[:], eq[:], rev_f[:], op=mult)
        rev_best = work.tile([P, 1], fp32)
        nc.vector.reduce_max(rev_best[:], val[:], axis=AX)

        idx_f = work.tile([P, 1], fp32)
        nc.vector.tensor_scalar(idx_f[:], rev_best[:], -1.0, float(K), op0=mult, op1=add)
        idx_i = work.tile([P, 1], i32)
        nc.vector.tensor_copy(idx_i[:], idx_f[:])

        # gather centroid rows (fp32, from the gathered on-device table)
        ecent = work.tile([P, D], fp32)
        nc.gpsimd.indirect_dma_start(
            out=ecent[:],
            out_offset=None,
            in_=cent_full[:],
            in_offset=bass.IndirectOffsetOnAxis(ap=idx_i[:, :1], axis=0),
        )

        # out*255 = (255*FREEDOM) * f + (255*(1-FREEDOM)) * ecent, as u8 on
        # the wire (host divides by 255; <=0.5 LSB from the writeout convert).
        # psm is already 255*m, so the f term only needs recip*FREEDOM.
        recip01 = work.tile([P, 1], fp32)
        nc.vector.tensor_scalar(recip01[:], recip[:], FREEDOM, None, op0=mult)
        t_free = outp.tile([P, D], fp32)
        nc.vector.tensor_scalar(t_free[:], psm[:, 0:D], recip01[:], None, op0=mult)
        o_sb = outp.tile([P, D], fp32)
        nc.vector.tensor_scalar(o_sb[:], ecent[:], 255.0 * (1.0 - FREEDOM), None, op0=mult)
        o_u8 = outp.tile([P, D], u8)
        nc.vector.tensor_tensor(o_u8[:], o_sb[:], t_free[:], op=add)
        nc.sync.dma_start(out[i * P : (i + 1) * P, :], o_u8[:])


def build_nc():
    nc = bacc.Bacc("TRN2", target_bir_lowering=False, debug=False)
    blob = nc.dram_tensor("blob", [S, C // 2 + D], u8, kind="ExternalInput")
    centp = nc.dram_tensor("centp", [KSH, D], fp32, kind="ExternalInput")
    out = nc.dram_tensor("out", [S, D], u8, kind="ExternalOutput")
    with tile.TileContext(nc) as tc:
        with ExitStack() as ctx:
            _body(ctx, tc, nc, blob, centp, out)
    nc.compile()
    return nc


def get_nc():
    if "nc" not in _nc_cache:
        _nc_cache["nc"] = build_nc()
    return _nc_cache["nc"]


def _get_host_fns():
    """jax-cpu jits for the host-side casts (numpy is ~100x slower here)."""
    if "host" not in _nc_cache:
        import jax
        import jax.numpy as jnp

        _ensure_jax_cache()
        cpu = jax.devices("cpu")[0]

        def _pack(x, e):  # -> [B, S, C/2 + D] u8: packed 4-bit seq ++ u8 emb shard
            q1 = (x[:, :, : C // 2] * 15.0 + 0.5).astype(jnp.uint8)
            q2 = (x[:, :, C // 2 :] * 15.0 + 0.5).astype(jnp.uint8)
            eq = (e * 255.0 + 0.5).astype(jnp.uint8).reshape(B, CSH, D)
            return jnp.concatenate([q1 | (q2 << 4), eq], axis=2)

        pack = jax.jit(_pack, device=cpu)
        tofp32 = jax.jit(lambda x: x.astype(jnp.float32) * (1.0 / 255.0), device=cpu)
        _nc_cache["host"] = (pack, tofp32)
    return _nc_cache["host"]


def kernel(concept_seq, concept_emb, centroid_emb, domain=None, **_ignored):
    import time as _time

    timing = _os.environ.get("K_TIME") == "1"
    t0 = _time.time()
    pack, tofp32 = _get_host_fns()
    blob = np.asarray(pack(np.asarray(concept_seq), np.asarray(concept_emb)))
    cent = np.ascontiguousarray(np.asarray(centroid_emb), dtype=np.float32)
    t1 = _time.time()
    nc = get_nc()
    in_maps = [
        {
            "blob": blob[b],
            "centp": cent[b * KSH : (b + 1) * KSH],
        }
        for b in range(B)
    ]
    trace = _os.environ.get("K_TRACE") == "1"
    t2 = _time.time()
    try:
        res = run_bass_kernel_spmd(nc, in_maps, core_ids=list(range(B)), trace=trace)
    except Exception:
        # transient device states (e.g. NRT_EXEC_UNIT_UNRECOVERABLE after an
        # interrupted run) clear on a fresh attempt
        res = run_bass_kernel_spmd(nc, in_maps, core_ids=list(range(B)), trace=trace)
    t3 = _time.time()
    _nc_cache["last_res"] = res
    out_u8 = np.stack([res.results[b]["out"] for b in range(B)], axis=0)
    result = np.asarray(tofp32(out_u8))
    if timing:
        print(
            f"[ktime] host_prep={t1-t0:.3f}s build={t2-t1:.3f}s "
            f"run={t3-t2:.3f}s post={_time.time()-t3:.3f}s",
            flush=True,
        )
    return result


if __name__ == "__main__":
    rng = np.random.default_rng(0)
    seq = rng.random((B, S, C), dtype=np.float32)
    emb = rng.random((C, D), dtype=np.float32)
    cent = rng.random((K, D), dtype=np.float32)
    got = kernel(seq, emb, cent, 0)
    cnt = seq.sum(-1, keepdims=True)
    cnt[cnt == 0] = 1
    f = (seq / cnt).reshape(-1, C) @ emb
    d2 = (f * f).sum(1, keepdims=True) - 2 * f @ cent.T + (cent * cent).sum(1)
    ec = cent[np.argmin(d2, 1)]
    ref = (FREEDOM * f + (1 - FREEDOM) * ec).reshape(B, S, D)
    rel = np.linalg.norm(got - ref) / np.linalg.norm(ref)
    print("rel err:", rel)


# revision 28
# speedup vs baseline: 1.0758x; 1.0758x over previous
"""ConceptEmbedding kernel for 8 Trainium2 NeuronCores.

Data-parallel over batch (B=8 -> 1 batch per core). Per core:
  m[s,d]   = sum_c seq[s,c] * emb[c,d]      (PE, bf16 inputs, fp32 psum)
  cnt[s]   = sum_c seq[s,c]                 (fused: ones column appended to emb rhs)
  f        = m / max(cnt,1)                 (e_free; u8 seq scale cancels in the ratio)
  idx[s]   = argmax_k (f . cent_k - 0.5*|cent_k|^2)   == argmin_k ||f - cent_k||^2
  out      = 0.1*f + 0.9*cent[idx]

The session bottleneck is the ~50-70 MB/s axon tunnel, not the device, so the
wire format is minimized: seq ships 4-bit quantized, two values per byte
(round(15*x); the scale cancels in m/cnt; column c' of the packed byte holds
column c' in the low nibble and column c'+C/2 in the high nibble, so the
device unpack keeps the original column order), the embedding table ships u8
(round(255*e), the 1/255 folded into the reciprocal stage) and sharded
1/8-per-core (AllGather on device rebuilds the full table), centroids ship
fp32 sharded 1/8-per-core, the K-k tie-break ramp is generated on device with
iota, and the output returns as u8 (x255). Host-side casts run through
jax-cpu jits - plain numpy elementwise on this host is ~100x slower.
"""

import os as _os
import sys
from contextlib import ExitStack

sys.path.insert(0, "/opt/trn_rl_repo")

# The per-call jax.jit inside run_bass_via_pjrt is a fresh closure, so every
# call misses jax's in-memory executable cache and re-runs the neuronx hook
# (~0.5s). The persistent compilation cache turns that into a disk hit.
_os.environ.setdefault("JAX_COMPILATION_CACHE_DIR", "/tmp/jaxcache")
_os.environ.setdefault("JAX_PERSISTENT_CACHE_MIN_COMPILE_TIME_SECS", "0")
_os.environ.setdefault("JAX_PERSISTENT_CACHE_MIN_ENTRY_SIZE_BYTES", "0")

import numpy as np

import concourse.bass as bass
import concourse.mybir as mybir
import concourse.tile as tile
from concourse import bacc
from concourse.bass_utils import run_bass_kernel_spmd
from concourse.masks import make_identity

B, S, C, D, K = 8, 1024, 8192, 256, 512
FREEDOM = 0.1
P = 128
CT = C // P  # 64 c-tiles
ST = S // P  # 8 s-tiles
KT = K // P  # 4 k-tiles
DH = D // P  # 2 d-halves
NCORE = 8
CSH = C // NCORE  # 1024 emb rows per core on the wire
KSH = K // NCORE  # 64 centroid rows per core on the wire

fp32 = mybir.dt.float32
bf16 = mybir.dt.bfloat16
i32 = mybir.dt.int32
u8 = mybir.dt.uint8

_nc_cache = {}


def _ensure_jax_cache():
    """config.update is authoritative even if jax was imported before us."""
    if "jaxcache" in _nc_cache:
        return
    import jax

    try:
        jax.config.update("jax_compilation_cache_dir", "/tmp/jaxcache")
        jax.config.update("jax_persistent_cache_min_compile_time_secs", 0.0)
        jax.config.update("jax_persistent_cache_min_entry_size_bytes", 0)
    except Exception:
        pass
    _nc_cache["jaxcache"] = True


def _body(ctx, tc, nc, seq, embp, centp, out):
    mult = mybir.AluOpType.mult
    add = mybir.AluOpType.add
    is_ge = mybir.AluOpType.is_ge
    is_equal = mybir.AluOpType.is_equal
    AX = mybir.AxisListType.X

    const = ctx.enter_context(tc.tile_pool(name="const", bufs=1))
    dram = ctx.enter_context(tc.tile_pool(name="dram", bufs=1, space="DRAM"))
    natq_pool = ctx.enter_context(tc.tile_pool(name="natq", bufs=2))
    nat_pool = ctx.enter_context(tc.tile_pool(name="nat", bufs=2))
    seqT_pool = ctx.enter_context(tc.tile_pool(name="seqT", bufs=2))
    work = ctx.enter_context(tc.tile_pool(name="work", bufs=3))
    outp = ctx.enter_context(tc.tile_pool(name="outp", bufs=3))
    ps_t = ctx.enter_context(tc.tile_pool(name="ps_t", bufs=2, space="PSUM"))
    ps_m = ctx.enter_context(tc.tile_pool(name="ps_m", bufs=2, space="PSUM"))
    ps_g = ctx.enter_context(tc.tile_pool(name="ps_g", bufs=2, space="PSUM"))
    ps_f = ctx.enter_context(tc.tile_pool(name="ps_f", bufs=2, space="PSUM"))

    # ---------------- gather the sharded tables on device ----------------
    groups = [list(range(NCORE))]
    emb_in = dram.tile([CSH, D], u8)
    emb_full = dram.tile([C, D], u8)
    nc.gpsimd.dma_start(emb_in[:], embp[:])
    nc.gpsimd.collective_compute(
        "AllGather",
        mybir.AluOpType.bypass,
        replica_groups=groups,
        ins=[emb_in.opt()],
        outs=[emb_full.opt()],
    )
    cent_in = dram.tile([KSH, D], fp32)
    cent_full = dram.tile([K, D], fp32)
    nc.gpsimd.dma_start(cent_in[:], centp[:])
    nc.gpsimd.collective_compute(
        "AllGather",
        mybir.AluOpType.bypass,
        replica_groups=groups,
        ins=[cent_in.opt()],
        outs=[cent_full.opt()],
    )

    # ---------------- constants ----------------
    ident = const.tile([P, P], bf16)
    make_identity(nc, ident[:])
    ident_f = const.tile([P, P], fp32)
    make_identity(nc, ident_f[:])

    # emb_aug[p, t, 0:256] = round(255*emb[t*128+p, :]) as bf16 (u8 values are
    # exact in bf16; the 1/255 is folded into the reciprocal stage);
    # col 256 = 1.0 (row count)
    emb_u8_sb = const.tile([P, CT, D], u8)
    nc.gpsimd.dma_start(
        out=emb_u8_sb[:],
        in_=emb_full[:].rearrange("(t p) d -> p t d", p=P),
    )
    emb_aug = const.tile([P, CT, D + 1], bf16)
    nc.vector.tensor_copy(emb_aug[:, :, 0:D], emb_u8_sb[:])
    nc.vector.memset(emb_aug[:, :, D : D + 1], 1.0)

    # centroids natural fp32; transposed fp32 centT[d, k] (the scoring path
    # must be fp32: bf16 jitter exceeds the argmin margins)
    cent_nat = const.tile([P, KT, D], fp32)
    nc.sync.dma_start(cent_nat[:], cent_full[:].rearrange("(t p) d -> p t d", p=P))

    centT = const.tile([P, DH, K], fp32)
    for t in range(KT):
        pst = ps_t.tile([P, 4, P], fp32, tag="tp")
        for dh in range(DH):
            nc.tensor.matmul(
                pst[:, dh, :],
                lhsT=cent_nat[:, t, dh * P : (dh + 1) * P],
                rhs=ident_f[:],
                start=True,
                stop=True,
            )
        nc.any.tensor_copy(centT[:, :, t * P : (t + 1) * P], pst[:, 0:DH, :])

    # negh_mat[s, k] = -0.5 * |cent_k|^2 (same row in every partition, fp32)
    sq = const.tile([P, KT, D], fp32)
    nc.vector.tensor_tensor(sq[:], cent_nat[:], cent_nat[:], op=mult)
    negh_col = const.tile([P, KT], fp32)
    nc.vector.tensor_reduce(negh_col[:], sq[:], axis=AX, op=add)
    negh_cols = const.tile([P, KT], fp32)
    nc.vector.tensor_scalar_mul(negh_cols[:], negh_col[:], -0.5)
    psh = ps_g.tile([P, K], fp32, tag="g")
    for t in range(KT):
        nc.tensor.matmul(
            psh[:, t * P : (t + 1) * P],
            lhsT=negh_cols[:, t : t + 1].to_broadcast([P, P]),
            rhs=ident_f[:],
            start=True,
            stop=True,
        )
    negh_mat = const.tile([P, K], fp32)
    nc.vector.tensor_copy(negh_mat[:], psh[:])

    # rev[p, k] = K - k (same in every partition), via on-device iota
    rev_i = const.tile([P, K], i32)
    nc.gpsimd.iota(rev_i[:], pattern=[[-1, K]], base=K, channel_multiplier=0)
    rev_f = const.tile([P, K], fp32)
    nc.vector.tensor_copy(rev_f[:], rev_i[:])

    # ---------------- main loop over s-tiles ----------------
    band = mybir.AluOpType.bitwise_and
    shr = mybir.AluOpType.logical_shift_right
    for i in range(ST):
        natq = natq_pool.tile([P, C // 2], u8)  # packed 4-bit seq rows
        nc.gpsimd.dma_start(natq[:], seq[i * P : (i + 1) * P, :])
        # unpack nibbles -> bf16 (exact for 0..15); low nibble = cols [0, C/2),
        # high nibble = cols [C/2, C)
        lo_u8 = nat_pool.tile([P, C // 2], u8, tag="lo8")
        hi_u8 = nat_pool.tile([P, C // 2], u8, tag="hi8")
        nc.vector.tensor_scalar(lo_u8[:], natq[:], 15, None, op0=band)
        nc.vector.tensor_scalar(hi_u8[:], natq[:], 4, None, op0=shr)
        nat = nat_pool.tile([P, C], bf16, tag="nat")
        nc.any.tensor_copy(nat[:, 0 : C // 2], lo_u8[:])
        nc.any.tensor_copy(nat[:, C // 2 : C], hi_u8[:])

        # transpose 64 c-tiles on PE: seqT[c_local, ct, s_local]
        seqT = seqT_pool.tile([P, CT, P], bf16)
        for g in range(CT // 4):
            pst = ps_t.tile([P, 4, P], fp32, tag="tp")
            for j in range(4):
                c = g * 4 + j
                nc.tensor.matmul(
                    pst[:, j, :],
                    lhsT=nat[:, c * P : (c + 1) * P],
                    rhs=ident[:],
                    start=True,
                    stop=True,
                )
            nc.any.tensor_copy(seqT[:, g * 4 : (g + 1) * 4, :], pst[:])

        # main accumulation: psm[s, 0:256] = (15*255)*m-ish, psm[s, 256] = 15*cnt;
        # the common 15 cancels in m/cnt, the 255 is divided out via recip255
        psm = ps_m.tile([P, D + 1], fp32)
        for c in range(CT):
            nc.tensor.matmul(
                psm[:],
                lhsT=seqT[:, c, :],
                rhs=emb_aug[:, c, :],
                start=(c == 0),
                stop=(c == CT - 1),
            )

        # cnt guard + reciprocal
        iszero = work.tile([P, 1], fp32)
        nc.vector.tensor_scalar(iszero[:], psm[:, D : D + 1], 0.0, None, op0=is_equal)
        cnt_adj = work.tile([P, 1], fp32)
        nc.vector.tensor_tensor(cnt_adj[:], psm[:, D : D + 1], iszero[:], op=add)
        recip = work.tile([P, 1], fp32)
        nc.vector.reciprocal(recip[:], cnt_adj[:])
        recip255 = work.tile([P, 1], fp32)
        nc.vector.tensor_scalar(recip255[:], recip[:], 1.0 / 255.0, None, op0=mult)

        # f = m / cnt / 255 (fp32 for the scoring path; /255 undoes the u8 emb)
        f_sb = work.tile([P, D], fp32)
        nc.vector.tensor_scalar(f_sb[:], psm[:, 0:D], recip255[:], None, op0=mult)

        # fT via PE transpose
        psf = ps_f.tile([P, DH, P], fp32)
        for dh in range(DH):
            nc.tensor.matmul(
                psf[:, dh, :],
                lhsT=f_sb[:, dh * P : (dh + 1) * P],
                rhs=ident_f[:],
                start=True,
                stop=True,
            )
        fT = work.tile([P, DH, P], fp32)
        nc.any.tensor_copy(fT[:], psf[:])

        # G[s, k] = f . cent_k, then add -0.5|cent_k|^2 on DVE (fp32 path)
        psg = ps_g.tile([P, K], fp32, tag="g")
        nc.tensor.matmul(psg[:], lhsT=fT[:, 0, :], rhs=centT[:, 0, :], start=True, stop=False)
        nc.tensor.matmul(psg[:], lhsT=fT[:, 1, :], rhs=centT[:, 1, :], start=False, stop=True)
        gsc = work.tile([P, K], fp32)
        nc.vector.tensor_tensor(gsc[:], psg[:], negh_mat[:], op=add)

        # argmax over k (first max index, matching jnp.argmin tie-break)
        mx = work.tile([P, 1], fp32)
        nc.vector.reduce_max(mx[:], gsc[:], axis=AX)
        eq = work.tile([P, K], bf16)
        nc.vector.tensor_scalar(eq[:], gsc[:], mx[:], None, op0=is_ge)
        val = work.tile([P, K], fp32)
        nc.vector.tensor_tensor(val[:], eq[:], rev_f[:], op=mult)
        rev_best = work.tile([P, 1], fp32)
        nc.vector.reduce_max(rev_best[:], val[:], axis=AX)

        idx_f = work.tile([P, 1], fp32)
        nc.vector.tensor_scalar(idx_f[:], rev_best[:], -1.0, float(K), op0=mult, op1=add)
        idx_i = work.tile([P, 1], i32)
        nc.vector.tensor_copy(idx_i[:], idx_f[:])

        # gather centroid rows (fp32, from the gathered on-device table)
        ecent = work.tile([P, D], fp32)
        nc.gpsimd.indirect_dma_start(
            out=ecent[:],
            out_offset=None,
            in_=cent_full[:],
            in_offset=bass.IndirectOffsetOnAxis(ap=idx_i[:, :1], axis=0),
        )

        # out*255 = (255*FREEDOM) * f + (255*(1-FREEDOM)) * ecent, as u8 on
        # the wire (host divides by 255; <=0.5 LSB from the writeout convert).
        # psm is already 255*m, so the f term only needs recip*FREEDOM.
        recip01 = work.tile([P, 1], fp32)
        nc.vector.tensor_scalar(recip01[:], recip[:], FREEDOM, None, op0=mult)
        t_free = outp.tile([P, D], fp32)
        nc.vector.tensor_scalar(t_free[:], psm[:, 0:D], recip01[:], None, op0=mult)
        o_sb = outp.tile([P, D], fp32)
        nc.vector.tensor_scalar(o_sb[:], ecent[:], 255.0 * (1.0 - FREEDOM), None, op0=mult)
        o_u8 = outp.tile([P, D], u8)
        nc.vector.tensor_tensor(o_u8[:], o_sb[:], t_free[:], op=add)
        nc.sync.dma_start(out[i * P : (i + 1) * P, :], o_u8[:])


def build_nc():
    nc = bacc.Bacc("TRN2", target_bir_lowering=False, debug=False)
    seq = nc.dram_tensor("seq", [S, C // 2], u8, kind="ExternalInput")
    embp = nc.dram_tensor("embp", [CSH, D], u8, kind="ExternalInput")
    centp = nc.dram_tensor("centp", [KSH, D], fp32, kind="ExternalInput")
    out = nc.dram_tensor("out", [S, D], u8, kind="ExternalOutput")
    with tile.TileContext(nc) as tc:
        with ExitStack() as ctx:
            _body(ctx, tc, nc, seq, embp, centp, out)
    nc.compile()
    return nc


def get_nc():
    if "nc" not in _nc_cache:
        _nc_cache["nc"] = build_nc()
    return _nc_cache["nc"]


def _get_host_fns():
    """jax-cpu jits for the host-side casts (numpy is ~100x slower here)."""
    if "host" not in _nc_cache:
        import jax
        import jax.numpy as jnp

        _ensure_jax_cache()
        cpu = jax.devices("cpu")[0]

        def _pack(x):  # [B,S,C] fp32 -> [B,S,C/2] u8, two 4-bit values/byte
            q1 = (x[:, :, : C // 2] * 15.0 + 0.5).astype(jnp.uint8)
            q2 = (x[:, :, C // 2 :] * 15.0 + 0.5).astype(jnp.uint8)
            return q1 | (q2 << 4)

        pack = jax.jit(_pack, device=cpu)
        tou8 = jax.jit(lambda x: (x * 255.0 + 0.5).astype(jnp.uint8), device=cpu)
        tofp32 = jax.jit(lambda x: x.astype(jnp.float32) * (1.0 / 255.0), device=cpu)
        _nc_cache["host"] = (pack, tou8, tofp32)
    return _nc_cache["host"]


def kernel(concept_seq, concept_emb, centroid_emb, domain=None, **_ignored):
    import time as _time

    timing = _os.environ.get("K_TIME") == "1"
    t0 = _time.time()
    pack, tou8, tofp32 = _get_host_fns()
    seq_q = np.asarray(pack(np.asarray(concept_seq)))
    emb_q = np.asarray(tou8(np.asarray(concept_emb)))
    cent = np.ascontiguousarray(np.asarray(centroid_emb), dtype=np.float32)
    t1 = _time.time()
    nc = get_nc()
    in_maps = [
        {
            "seq": seq_q[b],
            "embp": emb_q[b * CSH : (b + 1) * CSH],
            "centp": cent[b * KSH : (b + 1) * KSH],
        }
        for b in range(B)
    ]
    trace = _os.environ.get("K_TRACE") == "1"
    t2 = _time.time()
    try:
        res = run_bass_kernel_spmd(nc, in_maps, core_ids=list(range(B)), trace=trace)
    except Exception:
        # transient device states (e.g. NRT_EXEC_UNIT_UNRECOVERABLE after an
        # interrupted run) clear on a fresh attempt
        res = run_bass_kernel_spmd(nc, in_maps, core_ids=list(range(B)), trace=trace)
    t3 = _time.time()
    _nc_cache["last_res"] = res
    out_u8 = np.stack([res.results[b]["out"] for b in range(B)], axis=0)
    result = np.asarray(tofp32(out_u8))
    if timing:
        print(
            f"[ktime] host_prep={t1-t0:.3f}s build={t2-t1:.3f}s "
            f"run={t3-t2:.3f}s post={_time.time()-t3:.3f}s",
            flush=True,
        )
    return result


if __name__ == "__main__":
    rng = np.random.default_rng(0)
    seq = rng.random((B, S, C), dtype=np.float32)
    emb = rng.random((C, D), dtype=np.float32)
    cent = rng.random((K, D), dtype=np.float32)
    got = kernel(seq, emb, cent, 0)
    cnt = seq.sum(-1, keepdims=True)
    cnt[cnt == 0] = 1
    f = (seq / cnt).reshape(-1, C) @ emb
    d2 = (f * f).sum(1, keepdims=True) - 2 * f @ cent.T + (cent * cent).sum(1)
    ec = cent[np.argmin(d2, 1)]
    ref = (FREEDOM * f + (1 - FREEDOM) * ec).reshape(B, S, D)
    rel = np.linalg.norm(got - ref) / np.linalg.norm(ref)
    print("rel err:", rel)
